# revision 1
# baseline (speedup 1.0000x reference)
"""Multi-head attention (B=8, S=1024, D=1024, H=16, dk=dv=64) on 8 TRN2 cores.

Sharding: data-parallel over batch — core b computes batch element b end to
end; no collectives. Host-side prep transposes activations/weights into the
layouts TensorE needs (contraction dim on partitions); all matmuls run on
device.

Per-core dataflow (everything "T" = [feature, seq] layout):
  qT[i,s] = sum_d WQT[d,i] * XQT[d,s]        (fp32r matmuls, N=512)
  kT      likewise; v[s,c] natural layout (XVT stationary)
  per head h:
    scoresT[s2,s1] = sum_j kT_h[j,s2] * qT_h[j,s1]   (K=64)
    expT = exp(scoresT/8)  on ScalarE (scale imm), bf16
    PV:  lhsT = [v_h | ones] (65 cols)  ->  psum[0:64,:]=ctx_unnorm^T,
         psum[64,:]= softmax denominator (free via the ones column)
    ctxT[c,s1] = psum[jv,s1] * recip(denominator)[s1]
  out[s1,m] = sum_c ctxT[c,s1] * WfcT[c,m]   (fp32r)
"""

import os

import numpy as np

import concourse.bacc as bacc
import concourse.mybir as mybir
import concourse.tile as tile
from concourse.bass_utils import run_bass_kernel_spmd

_LDW_OPT = os.environ.get("KMHA_LDW_OPT", "0") == "1"


def _install_ldw_opt():
    """Let walrus elide redundant LDWEIGHTS (same stationary operand reused
    by consecutive matmuls). The repo default disables the pass; flip it for
    this kernel, where ~40% of matmuls reuse the previous weights."""
    import concourse.bass_utils as bu

    if getattr(bu, "_kmha_ldw_patched", False):
        return
    orig = bu.run_command

    def patched(argv, **kw):
        argv = ["--enable-ldw-opt=true" if a == "--enable-ldw-opt=false" else a
                for a in argv]
        return orig(argv, **kw)

    bu.run_command = patched
    bu._kmha_ldw_patched = True

S = 1024
D = 1024
H = 16
DK = 64
P = 128
NT = S // P          # 8 seq/feature tiles
NCH = 2              # 512-wide free-dim chunks
CH = S // NCH        # 512
F32 = mybir.dt.float32
F32R = mybir.dt.float32r
BF16 = mybir.dt.bfloat16
EXP = mybir.ActivationFunctionType.Exp

_CACHE = {}


def _build():
    nc = bacc.Bacc("TRN2", target_bir_lowering=False, debug=False)
    xqt = nc.dram_tensor("xqt", [D, S], BF16, kind="ExternalInput").ap()
    xkt = nc.dram_tensor("xkt", [D, S], BF16, kind="ExternalInput").ap()
    xvt = nc.dram_tensor("xvt", [D, S], BF16, kind="ExternalInput").ap()
    wqt = nc.dram_tensor("wqt", [D, D], BF16, kind="ExternalInput").ap()
    wkt = nc.dram_tensor("wkt", [D, D], BF16, kind="ExternalInput").ap()
    wvt = nc.dram_tensor("wvt", [D, D], BF16, kind="ExternalInput").ap()
    wft = nc.dram_tensor("wft", [D, D], BF16, kind="ExternalInput").ap()
    out = nc.dram_tensor("out", [S, D], F32, kind="ExternalOutput").ap()

    from contextlib import ExitStack

    with tile.TileContext(nc) as tc:
        with (
            tc.tile_pool(name="persist", bufs=1) as pp,
            tc.tile_pool(name="psum", bufs=2, space="PSUM") as psp,
        ):
            qT = [pp.tile([P, S], BF16, tag=f"qT{t}", name=f"qT{t}")
                  for t in range(NT)]
            kT = [pp.tile([P, S], BF16, tag=f"kT{t}", name=f"kT{t}")
                  for t in range(NT)]
            # v natural layout, ones column after each head (softmax denom)
            vpv = [pp.tile([P, H * (DK + 1)], BF16, tag=f"v{t}", name=f"v{t}")
                   for t in range(NT)]
            ctxT = [pp.tile([P, S], BF16, tag=f"c{t}", name=f"c{t}")
                    for t in range(NT)]

            # ---- v projection first (attention needs all of v) ----
            with ExitStack() as stk:
                # attn pool opens first and also hosts the q/k inputs, the
                # streamed q/k weight tiles, and (via tag-slot reuse after
                # the last qk projection) the fc weights and output staging
                ap_ = stk.enter_context(tc.tile_pool(name="attn", bufs=2))
                xtq = [ap_.tile([P, S], BF16, tag="xtq", name="xtq", bufs=8)
                       for _ in range(NT)]
                xtk = [ap_.tile([P, S], BF16, tag="xtk", name="xtk", bufs=8)
                       for _ in range(NT)]

                with tc.tile_pool(name="vld", bufs=8) as vp:
                    xts = [vp.tile([P, S], BF16, tag="xt", name="xt")
                           for _ in range(NT)]
                    ws = [vp.tile([P, D], BF16, tag="w", name="w")
                          for _ in range(NT)]
                    for t in range(NT):
                        nc.sync.dma_start(out=xts[t][:],
                                          in_=xvt[t * P:(t + 1) * P, :])
                        nc.sync.dma_start(out=ws[t][:],
                                          in_=wvt[t * P:(t + 1) * P, :])
                    # q/k input loads run behind the v loads, overlapping
                    # the v projection compute
                    for t in range(NT):
                        nc.sync.dma_start(out=xtq[t][:],
                                          in_=xqt[t * P:(t + 1) * P, :])
                        nc.sync.dma_start(out=xtk[t][:],
                                          in_=xkt[t * P:(t + 1) * P, :])
                    for s2 in range(NT):
                        pss = [psp.tile([P, CH], F32, tag="proj", name="proj")
                               for _ in range(NCH)]
                        for d in range(NT):
                            for c in range(NCH):
                                nc.tensor.matmul(
                                    pss[c][:],
                                    lhsT=xts[d][:, s2 * P:(s2 + 1) * P],
                                    rhs=ws[d][:, c * CH:(c + 1) * CH],
                                    start=(d == 0),
                                    stop=(d == NT - 1),
                                )
                        nc.vector.memset(
                            vpv[s2][:, 0:H * 65].rearrange(
                                "p (h x) -> p h x", x=65)[:, :, 64:65],
                            1.0,
                        )
                        for c in range(NCH):
                            dst_ap = vpv[s2][:, c * 520:(c + 1) * 520].rearrange(
                                "p (h x) -> p h x", x=65)[:, :, 0:64]
                            src_ap = pss[c][:].rearrange(
                                "p (h x) -> p h x", x=64)
                            nc.vector.tensor_copy(dst_ap, src_ap)

                # ---- q/k projections interleaved with attention head pairs
                # so the PE always has dense matmul work while ScalarE exps
                def qkproj(a):
                    for xts_, wsrc, dst in ((xtq, wqt, qT), (xtk, wkt, kT)):
                        pss = [psp.tile([P, CH], F32, tag="proj", name="proj")
                               for _ in range(NCH)]
                        for d in range(NT):
                            wt = ap_.tile([P, P], BF16, tag="wqk", name="wqk",
                                          bufs=18)
                            nc.sync.dma_start(
                                out=wt[:],
                                in_=wsrc[d * P:(d + 1) * P, a * P:(a + 1) * P])
                            for c in range(NCH):
                                nc.tensor.matmul(
                                    pss[c][:],
                                    lhsT=wt[:],
                                    rhs=xts_[d][:, c * CH:(c + 1) * CH],
                                    start=(d == 0),
                                    stop=(d == NT - 1),
                                )
                        for c in range(NCH):
                            nc.vector.tensor_copy(
                                dst[a][:, c * CH:(c + 1) * CH], pss[c][:])

                def scores(a):
                    # head pair (2a, 2a+1) on PE row strips 0-63 / 64-127;
                    # the two K=64 matmuls run concurrently in the array.
                    # exp tiles are [P, CH] per (chunk, head, s2) to halve
                    # the SBUF footprint.
                    exps = {}
                    for c in range(NCH):
                        for g in range(2):
                            exps[(c, g)] = [
                                ap_.tile([P, CH], BF16, tag=f"exp{g}_{t}",
                                         name=f"exp{g}_{t}")
                                for t in range(NT)]
                    for s2 in range(NT):
                        for c in range(NCH):
                            pss = [psp.tile([P, CH], F32, tag=f"sc{g}",
                                            name=f"sc{g}", bufs=2)
                                   for g in range(2)]
                            for g in range(2):
                                nc.tensor.matmul(
                                    pss[g][:],
                                    lhsT=kT[a][g * DK:(g + 1) * DK,
                                               s2 * P:(s2 + 1) * P],
                                    rhs=qT[a][g * DK:(g + 1) * DK,
                                              c * CH:(c + 1) * CH],
                                    start=True, stop=True,
                                    tile_position=(g * DK, 0),
                                )
                            for g in range(2):
                                nc.scalar.activation(
                                    exps[(c, g)][s2][:],
                                    pss[g][:], EXP, scale=0.125)
                    return exps

                def pv_phase(a, exps):
                    # PV accumulation; ctx_unnorm + denominators copied
                    # straight out of psum (frees the banks fast). Row (g,c)
                    # parks at partition 32*(2g+c) (SBUF bases must be
                    # multiples of 32); one batched reciprocal per pair.
                    rows = ap_.tile([97, CH], F32, tag="rows", name="rows")
                    for g in range(2):
                        h = 2 * a + g
                        pvs = [psp.tile([P, CH], F32, tag="pv", name="pv")
                               for _ in range(NCH)]
                        for s2 in range(NT):
                            for c in range(NCH):
                                nc.tensor.matmul(
                                    pvs[c][0:DK + 1, :],
                                    lhsT=vpv[s2][:, h * 65:(h + 1) * 65],
                                    rhs=exps[(c, g)][s2][:],
                                    start=(s2 == 0),
                                    stop=(s2 == NT - 1),
                                )
                        for c in range(NCH):
                            nc.vector.tensor_copy(
                                ctxT[a][g * DK:(g + 1) * DK,
                                        c * CH:(c + 1) * CH],
                                pvs[c][0:DK, :])
                            ri = 32 * (2 * g + c)
                            nc.vector.tensor_copy(
                                rows[ri:ri + 1, :],
                                pvs[c][DK:DK + 1, :])
                    rrec = ap_.tile([97, CH], F32, tag="rrec", name="rrec")
                    nc.vector.reciprocal(rrec[:], rows[:])
                    return rrec

                def norm_phase(a, rrec):
                    # stage each reciprocal row at partition 0 (the GpSimd
                    # broadcast ucode reads through core 0 = partitions
                    # 0-15), broadcast, then scale ctx in place on DVE
                    for g in range(2):
                        for c in range(NCH):
                            ri = 32 * (2 * g + c)
                            r0 = ap_.tile([1, CH], F32, tag="r0", name="r0",
                                          bufs=2)
                            nc.vector.tensor_copy(r0[:], rrec[ri:ri + 1, :])
                            rb = ap_.tile([P, CH], F32, tag="rb", name="rb",
                                          bufs=2)
                            nc.gpsimd.partition_broadcast(rb[:], r0[:])
                            sl = ctxT[a][g * DK:(g + 1) * DK,
                                         c * CH:(c + 1) * CH]
                            nc.vector.tensor_mul(
                                sl, sl, rb[g * DK:(g + 1) * DK, :])

                exps_hist = {}
                rrec_hist = {}
                for a in range(NT):
                    qkproj(a)
                    if a >= 2:
                        rrec_hist[a - 2] = pv_phase(a - 2,
                                                    exps_hist.pop(a - 2))
                    if a >= 1:
                        exps_hist[a - 1] = scores(a - 1)
                    if a >= 3:
                        norm_phase(a - 3, rrec_hist.pop(a - 3))

                # fc weights reuse the q/k input slots (same tag) freed by
                # the final projections — loads overlap the attention tail
                wf = [ap_.tile([P, S], BF16, tag="xtq", name="wf", bufs=8)
                      for _ in range(NT)]
                for t in range(NT):
                    nc.sync.dma_start(out=wf[t][:],
                                      in_=wft[t * P:(t + 1) * P, :])

                exps_hist[NT - 1] = scores(NT - 1)
                for a2 in (NT - 2, NT - 1):
                    rrec_hist[a2] = pv_phase(a2, exps_hist.pop(a2))
                    norm_phase(a2 - 1, rrec_hist.pop(a2 - 1))
                norm_phase(NT - 1, rrec_hist.pop(NT - 1))

                # ---- fc: out[s1, m] ----
                for s1 in range(NT):
                    pss = [psp.tile([P, CH], F32, tag="proj", name="proj")
                           for _ in range(NCH)]
                    for ct in range(NT):
                        for c in range(NCH):
                            nc.tensor.matmul(
                                pss[c][:],
                                lhsT=ctxT[ct][:, s1 * P:(s1 + 1) * P],
                                rhs=wf[ct][:, c * CH:(c + 1) * CH],
                                start=(ct == 0),
                                stop=(ct == NT - 1),
                            )
                    for c in range(NCH):
                        ob = ap_.tile([P, CH], F32, tag="xtk", name="ob",
                                      bufs=8)
                        nc.vector.tensor_copy(ob[:], pss[c][:])
                        nc.sync.dma_start(
                            out=out[s1 * P:(s1 + 1) * P, c * CH:(c + 1) * CH],
                            in_=ob[:],
                        )

    nc.compile()
    return nc


def run(inputs, trace=False):
    """inputs: dict with Q,K,V [8,1024,1024] and WQ,WK,WV,Wfc [1024,1024].
    Returns (out [8,1024,1024] fp32, exec_time_ns or None)."""
    if _LDW_OPT:
        _install_ldw_opt()
    if "nc" not in _CACHE:
        _CACHE["nc"] = _build()
    nc = _CACHE["nc"]

    import ml_dtypes
    bf16 = ml_dtypes.bfloat16
    f32 = np.float32
    wqt = np.ascontiguousarray(np.asarray(inputs["WQ"], dtype=f32).T.astype(bf16))
    wkt = np.ascontiguousarray(np.asarray(inputs["WK"], dtype=f32).T.astype(bf16))
    wvt = np.ascontiguousarray(np.asarray(inputs["WV"], dtype=f32).T.astype(bf16))
    wft = np.ascontiguousarray(np.asarray(inputs["Wfc"], dtype=f32).T.astype(bf16))
    Q = np.asarray(inputs["Q"], dtype=f32)
    K = np.asarray(inputs["K"], dtype=f32)
    V = np.asarray(inputs["V"], dtype=f32)

    in_maps = [
        {
            "xqt": np.ascontiguousarray(Q[b].T.astype(bf16)),
            "xkt": np.ascontiguousarray(K[b].T.astype(bf16)),
            "xvt": np.ascontiguousarray(V[b].T.astype(bf16)),
            "wqt": wqt, "wkt": wkt, "wvt": wvt, "wft": wft,
        }
        for b in range(8)
    ]
    res = run_bass_kernel_spmd(nc, in_maps, core_ids=list(range(8)), trace=trace)
    out = np.stack([res.results[b]["out"] for b in range(8)], axis=0)
    return out.astype(np.float32), res.exec_time_ns


def kernel(**inputs):
    return run(inputs, trace=False)[0]



# revision 11
# speedup vs baseline: 1.2124x; 1.2124x over previous
"""Multi-head attention (B=8, S=1024, D=1024, H=16, dk=dv=64) on 8 TRN2 cores.

Sharding: data-parallel over batch — core b computes batch element b end to
end; no collectives. Host-side prep transposes activations/weights into the
layouts TensorE needs (contraction dim on partitions); all matmuls run on
device.

Per-core dataflow (everything "T" = [feature, seq] layout):
  qT[i,s] = sum_d WQT[d,i] * XQT[d,s]        (bf16 matmuls, N=512)
  kT      likewise; v[s,c] natural layout (XVT stationary)
  per head pair a (heads 2a, 2a+1 on PE row strips 0-63 / 64-127):
    scoresT[s2,s1] = sum_j kT_h[j,s2] * qT_h[j,s1]  (K=64, N=1024,
      bf16 psum output, the two heads' matmuls run concurrently)
    one fused exp over [128, 2048] psum (both heads) on ScalarE -> bf16
    PV:  lhsT = [v_h | ones] (65 cols)  ->  psum[0:64,:]=ctx_unnorm^T,
         psum[64,:]= softmax denominator (free via the ones column)
    reciprocal_approx_fast on the denominator row straight out of psum,
    2x gpsimd partition_broadcast + one fused [128,1024] multiply
  out[s1,m] = sum_c ctxT[c,s1] * WfcT[c,m], split ct 0-3 / 4-6 / 7 so the
  early chunks overlap the attention pipeline (bf16 partials in SBUF).
"""

import numpy as np

import concourse.bacc as bacc
import concourse.mybir as mybir
import concourse.tile as tile
from concourse.bass_utils import run_bass_kernel_spmd

S = 1024
D = 1024
H = 16
DK = 64
P = 128
NT = S // P          # 8 seq/feature tiles
NCH = 2              # 512-wide free-dim chunks
CH = S // NCH        # 512
F32 = mybir.dt.float32
BF16 = mybir.dt.bfloat16
EXP = mybir.ActivationFunctionType.Exp

_CACHE = {}


def _build():
    nc = bacc.Bacc("TRN2", target_bir_lowering=False, debug=False)
    xqt = nc.dram_tensor("xqt", [D, S], BF16, kind="ExternalInput").ap()
    xkt = nc.dram_tensor("xkt", [D, S], BF16, kind="ExternalInput").ap()
    xvt = nc.dram_tensor("xvt", [D, S], BF16, kind="ExternalInput").ap()
    wqt = nc.dram_tensor("wqt", [D, D], BF16, kind="ExternalInput").ap()
    wkt = nc.dram_tensor("wkt", [D, D], BF16, kind="ExternalInput").ap()
    wvt = nc.dram_tensor("wvt", [D, D], BF16, kind="ExternalInput").ap()
    wft = nc.dram_tensor("wft", [D, D], BF16, kind="ExternalInput").ap()
    out = nc.dram_tensor("out", [S, D], F32, kind="ExternalOutput").ap()

    from contextlib import ExitStack

    with tile.TileContext(nc) as tc:
        with (
            tc.tile_pool(name="persist", bufs=1) as pp,
        ):
            qT = [pp.tile([P, S], BF16, tag=f"qT{t}", name=f"qT{t}")
                  for t in range(NT)]
            kT = [pp.tile([P, S], BF16, tag=f"kT{t}", name=f"kT{t}")
                  for t in range(NT)]
            # v natural layout, ones column after each head (softmax denom)
            vpv = [pp.tile([P, H * (DK + 1)], BF16, tag=f"v{t}", name=f"v{t}")
                   for t in range(NT)]
            ctxT = [pp.tile([P, S], BF16, tag=f"c{t}", name=f"c{t}")
                    for t in range(NT)]

            with ExitStack() as stk:
                ap_ = stk.enter_context(tc.tile_pool(name="attn", bufs=2))
                xtq = [ap_.tile([P, S], BF16, tag="xtq", name="xtq", bufs=8)
                       for _ in range(NT)]
                xtk = [ap_.tile([P, S], BF16, tag="xtk", name="xtk", bufs=8)
                       for _ in range(NT)]
                # fc weights for ct 0-3 get their own slots so the early fc
                # chunks can run while xtq is still live
                wf4 = [ap_.tile([P, S], BF16, tag="wf4", name="wf4", bufs=4)
                       for _ in range(4)]

                # ---- v projection first (attention needs all of v) ----
                with tc.tile_pool(name="vld", bufs=8) as vp, \
                     tc.tile_pool(name="vps", bufs=6, space="PSUM") as vpsp:
                    xts = [vp.tile([P, S], BF16, tag="xt", name="xt")
                           for _ in range(NT)]
                    ws = [vp.tile([P, D], BF16, tag="w", name="w")
                          for _ in range(NT)]
                    # v inputs chunked + d-ordered on the sync queue so the
                    # first contraction steps arrive early, the rest stream
                    # behind the matmuls
                    for t in range(NT):
                        for hh in range(2):
                            nc.sync.dma_start(
                                out=xts[t][hh * 64:(hh + 1) * 64, :],
                                in_=xvt[t * P + hh * 64:t * P + (hh + 1) * 64, :])
                        for hh in range(2):
                            nc.sync.dma_start(
                                out=ws[t][hh * 64:(hh + 1) * 64, :],
                                in_=wvt[t * P + hh * 64:t * P + (hh + 1) * 64, :])
                    # q/k inputs + early fc weights issue from the scalar
                    # queue in parallel (ScalarE is idle until the first exp)
                    for t in range(NT):
                        nc.scalar.dma_start(out=xtq[t][:],
                                            in_=xqt[t * P:(t + 1) * P, :])
                        nc.scalar.dma_start(out=xtk[t][:],
                                            in_=xkt[t * P:(t + 1) * P, :])
                    for ct in range(4):
                        nc.scalar.dma_start(out=wf4[ct][:],
                                            in_=wft[ct * P:(ct + 1) * P, :])

                    for s2 in range(NT):
                        pss = [vpsp.tile([P, CH], F32, tag="vp", name="vp")
                               for _ in range(NCH)]
                        for d in range(NT):
                            for c in range(NCH):
                                nc.tensor.matmul(
                                    pss[c][:],
                                    lhsT=xts[d][:, s2 * P:(s2 + 1) * P],
                                    rhs=ws[d][:, c * CH:(c + 1) * CH],
                                    start=(d == 0),
                                    stop=(d == NT - 1),
                                )
                        nc.vector.memset(
                            vpv[s2][:, 0:H * 65].rearrange(
                                "p (h x) -> p h x", x=65)[:, :, 64:65],
                            1.0,
                        )
                        for c in range(NCH):
                            dst_ap = vpv[s2][:, c * 520:(c + 1) * 520].rearrange(
                                "p (h x) -> p h x", x=65)[:, :, 0:64]
                            src_ap = pss[c][:].rearrange(
                                "p (h x) -> p h x", x=64)
                            nc.vector.tensor_copy(dst_ap, src_ap)

                # second SBUF pool for tags that only exist after the
                # v-load pool is gone (exp tiles, recip rows, fc partials) —
                # keeps the peak footprint under the SBUF limit
                ap2 = stk.enter_context(tc.tile_pool(name="attn2", bufs=2))
                # main psum pool for the attention loop: proj 2 + sc 4 + pv 2
                psp = stk.enter_context(
                    tc.tile_pool(name="psum", bufs=2, space="PSUM"))

                def qkproj(a):
                    for xts_, wsrc, dst in ((xtq, wqt, qT), (xtk, wkt, kT)):
                        pss = [psp.tile([P, CH], F32, tag="proj", name="proj")
                               for _ in range(NCH)]
                        for d in range(NT):
                            wt = ap_.tile([P, P], BF16, tag="wqk", name="wqk",
                                          bufs=12)
                            nc.sync.dma_start(
                                out=wt[:],
                                in_=wsrc[d * P:(d + 1) * P, a * P:(a + 1) * P])
                            for c in range(NCH):
                                nc.tensor.matmul(
                                    pss[c][:],
                                    lhsT=wt[:],
                                    rhs=xts_[d][:, c * CH:(c + 1) * CH],
                                    start=(d == 0),
                                    stop=(d == NT - 1),
                                )
                        for c in range(NCH):
                            nc.vector.tensor_copy(
                                dst[a][:, c * CH:(c + 1) * CH], pss[c][:])

                def scores(a):
                    # per (s2, c): one fp32 psum tile [128, 1024] spanning 2
                    # banks; the two heads' K=64 matmuls (N=512 each, row
                    # strips 0-63 / 64-127) run concurrently and each fill
                    # one bank; a single fused exp [128, 1024] reads both.
                    exps = []
                    for s2 in range(NT):
                        ecs = []
                        for c in range(NCH):
                            sc = psp.tile([P, S], F32, tag="sc", name="sc")
                            for g in range(2):
                                nc.tensor.matmul(
                                    sc[:, g * CH:(g + 1) * CH],
                                    lhsT=kT[a][g * DK:(g + 1) * DK,
                                               s2 * P:(s2 + 1) * P],
                                    rhs=qT[a][g * DK:(g + 1) * DK,
                                              c * CH:(c + 1) * CH],
                                    start=True, stop=True,
                                    tile_position=(g * DK, 0),
                                )
                            ec = ap2.tile([P, S], BF16, tag=f"e{s2}c{c}",
                                          name=f"e{s2}c{c}")
                            nc.scalar.activation(ec[:], sc[:], EXP,
                                                 scale=0.125)
                            ecs.append(ec)
                        exps.append(ecs)
                    return exps

                def pv_phase(a, exps):
                    # PV accumulation; ctx_unnorm copied straight out of
                    # psum; the denominator row feeds reciprocal_approx_fast
                    # directly (no gather), results land in r0 at
                    # partition 0 laid out [g0c0|g0c1|g1c0|g1c1].
                    r0 = ap2.tile([1, 2 * S], F32, tag="r0", name="r0",
                                  bufs=1)
                    for g in range(2):
                        h = 2 * a + g
                        pvs = [psp.tile([P, CH], F32, tag="pv", name="pv")
                               for _ in range(NCH)]
                        for c in range(NCH):
                            for s2 in range(NT):
                                nc.tensor.matmul(
                                    pvs[c][0:DK + 1, :],
                                    lhsT=vpv[s2][:, h * 65:(h + 1) * 65],
                                    rhs=exps[s2][c][:, g * CH:(g + 1) * CH],
                                    start=(s2 == 0),
                                    stop=(s2 == NT - 1),
                                )
                        for c in range(NCH):
                            nc.vector.tensor_copy(
                                ctxT[a][g * DK:(g + 1) * DK,
                                        c * CH:(c + 1) * CH],
                                pvs[c][0:DK, :])
                            nc.vector.tensor_copy(
                                r0[0:1, g * S + c * CH:
                                   g * S + (c + 1) * CH],
                                pvs[c][DK:DK + 1, :])
                    nc.vector.reciprocal_approx_fast(out=r0[:], in_=r0[:])
                    return r0

                def norm_phase(a, r0):
                    # broadcast recip rows across partitions (one per head)
                    # and scale ctxT with one multiply per head
                    for g in range(2):
                        rb = ap2.tile([P, S], F32, tag="rb", name="rb",
                                      bufs=1)
                        nc.gpsimd.partition_broadcast(
                            rb[:], r0[0:1, g * S:(g + 1) * S])
                        sl = ctxT[a][g * DK:(g + 1) * DK, :]
                        nc.vector.tensor_mul(sl, sl, rb[g * DK:(g + 1) * DK, :])

                # fc partial tiles (bf16) hold ct0-3, later += ct4-6
                fcp = [ap2.tile([P, CH], BF16, tag=f"fp{i}", name=f"fp{i}",
                                bufs=1)
                       for i in range(2 * NT)]

                def fc_chunk(s1, c, cts, wfs, first):
                    pss = psp.tile([P, CH], F32, tag="proj", name="proj")
                    for i, ct in enumerate(cts):
                        nc.tensor.matmul(
                            pss[:],
                            lhsT=ctxT[ct][:, s1 * P:(s1 + 1) * P],
                            rhs=wfs[ct][:, c * CH:(c + 1) * CH],
                            start=(i == 0),
                            stop=(i == len(cts) - 1),
                        )
                    if first:
                        nc.vector.tensor_copy(fcp[s1 * NCH + c][:], pss[:])
                    else:
                        nc.vector.tensor_add(
                            fcp[s1 * NCH + c][:], fcp[s1 * NCH + c][:],
                            pss[:])

                exps_hist = None
                for a in range(NT):
                    qkproj(a)
                    if a >= 2:
                        r0 = pv_phase(a - 2, exps_hist)
                        norm_phase(a - 2, r0)
                    if a >= 1:
                        exps_hist = scores(a - 1)
                    # fc for ct0-3 interleaves with the last two iterations
                    if a == 6:
                        for s1 in range(4):
                            for c in range(NCH):
                                fc_chunk(s1, c, [0, 1, 2, 3], wf4, True)
                    if a == 7:
                        for s1 in range(4, NT):
                            for c in range(NCH):
                                fc_chunk(s1, c, [0, 1, 2, 3], wf4, True)

                # fc weights for ct 4-7 reuse the q-input slots freed after
                # the last projection
                wf = {}
                for ct in range(4, NT):
                    t_ = ap_.tile([P, S], BF16, tag="xtq", name="wf", bufs=8)
                    nc.sync.dma_start(out=t_[:],
                                      in_=wft[ct * P:(ct + 1) * P, :])
                    wf[ct] = t_

                # drain: sc(7); pv(6)+norm(6); fc ct4-6; pv(7)+norm(7);
                # fc ct7 + final add + store
                exps_last = scores(NT - 1)
                r0 = pv_phase(NT - 2, exps_hist)
                norm_phase(NT - 2, r0)
                for s1 in range(NT):
                    for c in range(NCH):
                        fc_chunk(s1, c, [4, 5, 6], wf, False)
                r0 = pv_phase(NT - 1, exps_last)
                norm_phase(NT - 1, r0)

                for s1 in range(NT):
                    for c in range(NCH):
                        pss = psp.tile([P, CH], F32, tag="proj", name="proj")
                        nc.tensor.matmul(
                            pss[:],
                            lhsT=ctxT[NT - 1][:, s1 * P:(s1 + 1) * P],
                            rhs=wf[NT - 1][:, c * CH:(c + 1) * CH],
                            start=True, stop=True,
                        )
                        ob = ap_.tile([P, CH], F32, tag="xtk", name="ob",
                                      bufs=8)
                        nc.vector.tensor_add(
                            ob[:], fcp[s1 * NCH + c][:], pss[:])
                        for hh in range(2):
                            eng = nc.sync if hh == 0 else nc.scalar
                            eng.dma_start(
                                out=out[s1 * P + hh * 64:
                                        s1 * P + (hh + 1) * 64,
                                        c * CH:(c + 1) * CH],
                                in_=ob[hh * 64:(hh + 1) * 64, :],
                            )

    nc.compile()
    return nc


def run(inputs, trace=False):
    """inputs: dict with Q,K,V [8,1024,1024] and WQ,WK,WV,Wfc [1024,1024].
    Returns (out [8,1024,1024] fp32, exec_time_ns or None)."""
    if "nc" not in _CACHE:
        _CACHE["nc"] = _build()
    nc = _CACHE["nc"]

    import ml_dtypes
    bf16 = ml_dtypes.bfloat16
    f32 = np.float32
    wqt = np.ascontiguousarray(np.asarray(inputs["WQ"], dtype=f32).T.astype(bf16))
    wkt = np.ascontiguousarray(np.asarray(inputs["WK"], dtype=f32).T.astype(bf16))
    wvt = np.ascontiguousarray(np.asarray(inputs["WV"], dtype=f32).T.astype(bf16))
    wft = np.ascontiguousarray(np.asarray(inputs["Wfc"], dtype=f32).T.astype(bf16))
    Q = np.asarray(inputs["Q"], dtype=f32)
    K = np.asarray(inputs["K"], dtype=f32)
    V = np.asarray(inputs["V"], dtype=f32)

    in_maps = [
        {
            "xqt": np.ascontiguousarray(Q[b].T.astype(bf16)),
            "xkt": np.ascontiguousarray(K[b].T.astype(bf16)),
            "xvt": np.ascontiguousarray(V[b].T.astype(bf16)),
            "wqt": wqt, "wkt": wkt, "wvt": wvt, "wft": wft,
        }
        for b in range(8)
    ]
    res = run_bass_kernel_spmd(nc, in_maps, core_ids=list(range(8)), trace=trace)
    out = np.stack([res.results[b]["out"] for b in range(8)], axis=0)
    return out.astype(np.float32), res.exec_time_ns


def kernel(**inputs):
    return run(inputs, trace=False)[0]


# revision 12
# speedup vs baseline: 1.2572x; 1.0369x over previous
"""Multi-head attention (B=8, S=1024, D=1024, H=16, dk=dv=64) on 8 TRN2 cores.

Sharding: data-parallel over batch — core b computes batch element b end to
end; no collectives. Host-side prep transposes activations/weights into the
layouts TensorE needs (contraction dim on partitions); all matmuls run on
device.

Per-core dataflow (everything "T" = [feature, seq] layout):
  qT[i,s] = sum_d WQT[d,i] * XQT[d,s]        (bf16 matmuls, N=512)
  kT      likewise; v[s,c] natural layout (XVT stationary)
  per head pair a (heads 2a, 2a+1 on PE row strips 0-63 / 64-127):
    scoresT[s2,s1] = sum_j kT_h[j,s2] * qT_h[j,s1]  (K=64, N=1024,
      bf16 psum output, the two heads' matmuls run concurrently)
    one fused exp over [128, 2048] psum (both heads) on ScalarE -> bf16
    PV:  lhsT = [v_h | ones] (65 cols)  ->  psum[0:64,:]=ctx_unnorm^T,
         psum[64,:]= softmax denominator (free via the ones column)
    reciprocal_approx_fast on the denominator row straight out of psum,
    2x gpsimd partition_broadcast + one fused [128,1024] multiply
  out[s1,m] = sum_c ctxT[c,s1] * WfcT[c,m], split ct 0-3 / 4-6 / 7 so the
  early chunks overlap the attention pipeline (bf16 partials in SBUF).
"""

import numpy as np

import concourse.bacc as bacc
import concourse.mybir as mybir
import concourse.tile as tile
from concourse.bass_utils import run_bass_kernel_spmd

S = 1024
D = 1024
H = 16
DK = 64
P = 128
NT = S // P          # 8 seq/feature tiles
NCH = 2              # 512-wide free-dim chunks
CH = S // NCH        # 512
F32 = mybir.dt.float32
BF16 = mybir.dt.bfloat16
EXP = mybir.ActivationFunctionType.Exp

_CACHE = {}


def _build():
    nc = bacc.Bacc("TRN2", target_bir_lowering=False, debug=False)
    xqt = nc.dram_tensor("xqt", [D, S], BF16, kind="ExternalInput").ap()
    xkt = nc.dram_tensor("xkt", [D, S], BF16, kind="ExternalInput").ap()
    xvt = nc.dram_tensor("xvt", [D, S], BF16, kind="ExternalInput").ap()
    wqt = nc.dram_tensor("wqt", [D, D], BF16, kind="ExternalInput").ap()
    wkt = nc.dram_tensor("wkt", [D, D], BF16, kind="ExternalInput").ap()
    wvt = nc.dram_tensor("wvt", [D, D], BF16, kind="ExternalInput").ap()
    wft = nc.dram_tensor("wft", [D, D], BF16, kind="ExternalInput").ap()
    out = nc.dram_tensor("out", [S, D], F32, kind="ExternalOutput").ap()

    from contextlib import ExitStack

    with tile.TileContext(nc) as tc:
        with (
            tc.tile_pool(name="persist", bufs=1) as pp,
        ):
            # v natural layout, ones column after each head (softmax denom)
            vpv = [pp.tile([P, H * (DK + 1)], BF16, tag=f"v{t}", name=f"v{t}")
                   for t in range(NT)]
            ctxT = [pp.tile([P, S], BF16, tag=f"c{t}", name=f"c{t}")
                    for t in range(NT)]

            with ExitStack() as stk:
                ap_ = stk.enter_context(tc.tile_pool(name="attn", bufs=2))
                xtq = [ap_.tile([P, S], BF16, tag="xtq", name="xtq", bufs=8)
                       for _ in range(NT)]
                xtk = [ap_.tile([P, S], BF16, tag="xtk", name="xtk", bufs=8)
                       for _ in range(NT)]
                # fc weights for ct 0-3 get their own slots so the early fc
                # chunks can run while xtq is still live
                wf4 = [ap_.tile([P, S], BF16, tag="wf4", name="wf4", bufs=4)
                       for _ in range(4)]

                # ---- v projection first (attention needs all of v) ----
                with tc.tile_pool(name="vld", bufs=8) as vp, \
                     tc.tile_pool(name="vps", bufs=8, space="PSUM") as vpsp:
                    xts = [vp.tile([P, S], BF16, tag="xt", name="xt")
                           for _ in range(NT)]
                    ws = [vp.tile([P, D], BF16, tag="w", name="w")
                          for _ in range(NT)]
                    # v inputs chunked + d-ordered, split across BOTH
                    # hwdge issue queues (sync + scalar) so they land first;
                    # q/k follow, fc weights last (needed only at iter 6)
                    for t in range(NT):
                        nc.sync.dma_start(
                            out=xts[t][0:64, :],
                            in_=xvt[t * P:t * P + 64, :])
                        nc.scalar.dma_start(
                            out=xts[t][64:128, :],
                            in_=xvt[t * P + 64:(t + 1) * P, :])
                        nc.sync.dma_start(
                            out=ws[t][0:64, :],
                            in_=wvt[t * P:t * P + 64, :])
                        nc.scalar.dma_start(
                            out=ws[t][64:128, :],
                            in_=wvt[t * P + 64:(t + 1) * P, :])
                    for t in range(NT):
                        enq = nc.sync if t % 2 == 0 else nc.scalar
                        enk = nc.scalar if t % 2 == 0 else nc.sync
                        enq.dma_start(out=xtq[t][:],
                                      in_=xqt[t * P:(t + 1) * P, :])
                        enk.dma_start(out=xtk[t][:],
                                      in_=xkt[t * P:(t + 1) * P, :])
                    for ct in range(4):
                        nc.scalar.dma_start(out=wf4[ct][:],
                                            in_=wft[ct * P:(ct + 1) * P, :])

                    for s2 in range(NT):
                        pss = [vpsp.tile([P, CH], F32, tag="vp", name="vp")
                               for _ in range(NCH)]
                        for d in range(NT):
                            for c in range(NCH):
                                nc.tensor.matmul(
                                    pss[c][:],
                                    lhsT=xts[d][:, s2 * P:(s2 + 1) * P],
                                    rhs=ws[d][:, c * CH:(c + 1) * CH],
                                    start=(d == 0),
                                    stop=(d == NT - 1),
                                )
                        nc.vector.memset(
                            vpv[s2][:, 0:H * 65].rearrange(
                                "p (h x) -> p h x", x=65)[:, :, 64:65],
                            1.0,
                        )
                        for c in range(NCH):
                            dst_ap = vpv[s2][:, c * 520:(c + 1) * 520].rearrange(
                                "p (h x) -> p h x", x=65)[:, :, 0:64]
                            src_ap = pss[c][:].rearrange(
                                "p (h x) -> p h x", x=64)
                            nc.vector.tensor_copy(dst_ap, src_ap)

                # second SBUF pool for tags that only exist after the
                # v-load pool is gone (exp tiles, recip rows, fc partials) —
                # keeps the peak footprint under the SBUF limit
                ap2 = stk.enter_context(tc.tile_pool(name="attn2", bufs=2))
                # main psum pool for the attention loop: proj 2 + sc 4 + pv 2
                psp = stk.enter_context(
                    tc.tile_pool(name="psum", bufs=2, space="PSUM"))

                def qkproj(a):
                    # q/k head-pair tiles rotate (lifetime: this iteration's
                    # projection + next iteration's scores)
                    outs = []
                    for xts_, wsrc, tg in ((xtq, wqt, "qTr"), (xtk, wkt, "kTr")):
                        dst = pp.tile([P, S], BF16, tag=tg, name=tg, bufs=3)
                        pss = [psp.tile([P, CH], F32, tag="proj", name="proj")
                               for _ in range(NCH)]
                        for d in range(NT):
                            wt = ap_.tile([P, P], BF16, tag="wqk", name="wqk",
                                          bufs=12)
                            nc.sync.dma_start(
                                out=wt[:],
                                in_=wsrc[d * P:(d + 1) * P, a * P:(a + 1) * P])
                            for c in range(NCH):
                                nc.tensor.matmul(
                                    pss[c][:],
                                    lhsT=wt[:],
                                    rhs=xts_[d][:, c * CH:(c + 1) * CH],
                                    start=(d == 0),
                                    stop=(d == NT - 1),
                                )
                        for c in range(NCH):
                            nc.vector.tensor_copy(
                                dst[:, c * CH:(c + 1) * CH], pss[c][:])
                        outs.append(dst)
                    return outs

                def scores(qk):
                    qTa, kTa = qk
                    # per (s2, c): one fp32 psum tile [128, 1024] spanning 2
                    # banks; the two heads' K=64 matmuls (N=512 each, row
                    # strips 0-63 / 64-127) run concurrently and each fill
                    # one bank; a single fused exp [128, 1024] reads both.
                    exps = []
                    for s2 in range(NT):
                        ecs = []
                        for c in range(NCH):
                            sc = psp.tile([P, S], F32, tag="sc", name="sc")
                            for g in range(2):
                                nc.tensor.matmul(
                                    sc[:, g * CH:(g + 1) * CH],
                                    lhsT=kTa[g * DK:(g + 1) * DK,
                                             s2 * P:(s2 + 1) * P],
                                    rhs=qTa[g * DK:(g + 1) * DK,
                                            c * CH:(c + 1) * CH],
                                    start=True, stop=True,
                                    tile_position=(g * DK, 0),
                                )
                            ec = ap2.tile([P, S], BF16, tag=f"e{s2}c{c}",
                                          name=f"e{s2}c{c}")
                            nc.scalar.activation(ec[:], sc[:], EXP,
                                                 scale=0.125)
                            ecs.append(ec)
                        exps.append(ecs)
                    return exps

                def pv_phase(a, exps, on_act=False):
                    # PV accumulation; ctx_unnorm copied straight out of
                    # psum; the denominator row feeds reciprocal_approx_fast
                    # directly (no gather), results land in r0 at
                    # partition 0 laid out [g0c0|g0c1|g1c0|g1c1].
                    r0 = ap2.tile([1, 2 * S], F32, tag="r0", name="r0",
                                  bufs=2)
                    for g in range(2):
                        h = 2 * a + g
                        pvs = [psp.tile([P, CH], F32, tag="pv", name="pv")
                               for _ in range(NCH)]
                        for c in range(NCH):
                            for s2 in range(NT):
                                nc.tensor.matmul(
                                    pvs[c][0:DK + 1, :],
                                    lhsT=vpv[s2][:, h * 65:(h + 1) * 65],
                                    rhs=exps[s2][c][:, g * CH:(g + 1) * CH],
                                    start=(s2 == 0),
                                    stop=(s2 == NT - 1),
                                )
                        for c in range(NCH):
                            cp = nc.scalar.copy if on_act else \
                                nc.vector.tensor_copy
                            cp(ctxT[a][g * DK:(g + 1) * DK,
                                       c * CH:(c + 1) * CH],
                               pvs[c][0:DK, :])
                            cp(r0[0:1, g * S + c * CH:
                                  g * S + (c + 1) * CH],
                               pvs[c][DK:DK + 1, :])
                    nc.vector.reciprocal_approx_fast(out=r0[:], in_=r0[:])
                    return r0

                def norm_phase(a, r0):
                    # broadcast recip rows across partitions (one per head)
                    # and scale ctxT with one multiply per head
                    for g in range(2):
                        rb = ap2.tile([P, S], F32, tag="rb", name="rb",
                                      bufs=2)
                        nc.gpsimd.partition_broadcast(
                            rb[:], r0[0:1, g * S:(g + 1) * S])
                        sl = ctxT[a][g * DK:(g + 1) * DK, :]
                        nc.vector.tensor_mul(sl, sl, rb[g * DK:(g + 1) * DK, :])

                # fc partial tiles (bf16) hold ct0-3, later += ct4-6
                fcp = [ap2.tile([P, CH], BF16, tag=f"fp{i}", name=f"fp{i}",
                                bufs=1)
                       for i in range(2 * NT)]

                def fc_chunk(s1, c, cts, wfs, first):
                    pss = psp.tile([P, CH], F32, tag="proj", name="proj")
                    for i, ct in enumerate(cts):
                        nc.tensor.matmul(
                            pss[:],
                            lhsT=ctxT[ct][:, s1 * P:(s1 + 1) * P],
                            rhs=wfs[ct][:, c * CH:(c + 1) * CH],
                            start=(i == 0),
                            stop=(i == len(cts) - 1),
                        )
                    if first:
                        nc.vector.tensor_copy(fcp[s1 * NCH + c][:], pss[:])
                    else:
                        nc.vector.tensor_add(
                            fcp[s1 * NCH + c][:], fcp[s1 * NCH + c][:],
                            pss[:])

                exps_hist = None
                qk_hist = None
                for a in range(NT):
                    qk_new = qkproj(a)
                    if a >= 2:
                        r0 = pv_phase(a - 2, exps_hist)
                        norm_phase(a - 2, r0)
                    if a >= 1:
                        exps_hist = scores(qk_hist)
                    qk_hist = qk_new
                    # fc for ct0-3 interleaves with the last two iterations
                    if a == 6:
                        for s1 in range(4):
                            for c in range(NCH):
                                fc_chunk(s1, c, [0, 1, 2, 3], wf4, True)
                    if a == 7:
                        for s1 in range(4, NT):
                            for c in range(NCH):
                                fc_chunk(s1, c, [0, 1, 2, 3], wf4, True)

                # fc weights for ct 4-7 reuse the q-input slots freed after
                # the last projection
                wf = {}
                for ct in range(4, NT):
                    t_ = ap_.tile([P, S], BF16, tag="xtq", name="wf", bufs=8)
                    nc.sync.dma_start(out=t_[:],
                                      in_=wft[ct * P:(ct + 1) * P, :])
                    wf[ct] = t_

                # drain: sc(7); pv(6)+norm(6); fc ct4-6; pv(7)+norm(7);
                # fc ct7 + final add + store
                exps_last = scores(qk_hist)
                r0 = pv_phase(NT - 2, exps_hist)
                norm_phase(NT - 2, r0)
                for s1 in range(NT):
                    for c in range(NCH):
                        fc_chunk(s1, c, [4, 5, 6], wf, False)
                r0 = pv_phase(NT - 1, exps_last, on_act=True)
                norm_phase(NT - 1, r0)

                for s1 in range(NT):
                    for c in range(NCH):
                        pss = psp.tile([P, CH], F32, tag="proj", name="proj")
                        nc.tensor.matmul(
                            pss[:],
                            lhsT=ctxT[NT - 1][:, s1 * P:(s1 + 1) * P],
                            rhs=wf[NT - 1][:, c * CH:(c + 1) * CH],
                            start=True, stop=True,
                        )
                        ob = ap_.tile([P, CH], F32, tag="xtk", name="ob",
                                      bufs=8)
                        nc.vector.tensor_add(
                            ob[:], fcp[s1 * NCH + c][:], pss[:])
                        for hh in range(2):
                            eng = nc.sync if hh == 0 else nc.scalar
                            eng.dma_start(
                                out=out[s1 * P + hh * 64:
                                        s1 * P + (hh + 1) * 64,
                                        c * CH:(c + 1) * CH],
                                in_=ob[hh * 64:(hh + 1) * 64, :],
                            )

    nc.compile()
    return nc


def run(inputs, trace=False):
    """inputs: dict with Q,K,V [8,1024,1024] and WQ,WK,WV,Wfc [1024,1024].
    Returns (out [8,1024,1024] fp32, exec_time_ns or None)."""
    if "nc" not in _CACHE:
        _CACHE["nc"] = _build()
    nc = _CACHE["nc"]

    import ml_dtypes
    bf16 = ml_dtypes.bfloat16
    f32 = np.float32
    wqt = np.ascontiguousarray(np.asarray(inputs["WQ"], dtype=f32).T.astype(bf16))
    wkt = np.ascontiguousarray(np.asarray(inputs["WK"], dtype=f32).T.astype(bf16))
    wvt = np.ascontiguousarray(np.asarray(inputs["WV"], dtype=f32).T.astype(bf16))
    wft = np.ascontiguousarray(np.asarray(inputs["Wfc"], dtype=f32).T.astype(bf16))
    Q = np.asarray(inputs["Q"], dtype=f32)
    K = np.asarray(inputs["K"], dtype=f32)
    V = np.asarray(inputs["V"], dtype=f32)

    in_maps = [
        {
            "xqt": np.ascontiguousarray(Q[b].T.astype(bf16)),
            "xkt": np.ascontiguousarray(K[b].T.astype(bf16)),
            "xvt": np.ascontiguousarray(V[b].T.astype(bf16)),
            "wqt": wqt, "wkt": wkt, "wvt": wvt, "wft": wft,
        }
        for b in range(8)
    ]
    res = run_bass_kernel_spmd(nc, in_maps, core_ids=list(range(8)), trace=trace)
    out = np.stack([res.results[b]["out"] for b in range(8)], axis=0)
    return out.astype(np.float32), res.exec_time_ns


def kernel(**inputs):
    return run(inputs, trace=False)[0]


# revision 13
# speedup vs baseline: 1.3472x; 1.0716x over previous
"""Multi-head attention (B=8, S=1024, D=1024, H=16, dk=dv=64) on 8 TRN2 cores.

Sharding: data-parallel over batch — core b computes batch element b end to
end; no collectives. Host-side prep transposes activations/weights into the
layouts TensorE needs (contraction dim on partitions); all matmuls run on
device.

Per-core dataflow (everything "T" = [feature, seq] layout):
  qT[i,s] = sum_d WQT[d,i] * XQT[d,s]        (bf16 matmuls, N=512)
  kT      likewise; v[s,c] natural layout (XVT stationary)
  per head pair a (heads 2a, 2a+1 on PE row strips 0-63 / 64-127):
    scoresT[s2,s1] = sum_j kT_h[j,s2] * qT_h[j,s1]  (K=64, N=1024,
      bf16 psum output, the two heads' matmuls run concurrently)
    one fused exp over [128, 2048] psum (both heads) on ScalarE -> bf16
    PV:  lhsT = [v_h | ones] (65 cols)  ->  psum[0:64,:]=ctx_unnorm^T,
         psum[64,:]= softmax denominator (free via the ones column)
    reciprocal_approx_fast on the denominator row straight out of psum,
    2x gpsimd partition_broadcast + one fused [128,1024] multiply
  out[s1,m] = sum_c ctxT[c,s1] * WfcT[c,m], split ct 0-3 / 4-6 / 7 so the
  early chunks overlap the attention pipeline (bf16 partials in SBUF).
"""

import numpy as np

import concourse.bacc as bacc
import concourse.mybir as mybir
import concourse.tile as tile
from concourse.bass_utils import run_bass_kernel_spmd

S = 1024
D = 1024
H = 16
DK = 64
P = 128
NT = S // P          # 8 seq/feature tiles
NCH = 2              # 512-wide free-dim chunks
CH = S // NCH        # 512
F32 = mybir.dt.float32
BF16 = mybir.dt.bfloat16
EXP = mybir.ActivationFunctionType.Exp

_CACHE = {}


def _build():
    nc = bacc.Bacc("TRN2", target_bir_lowering=False, debug=False)
    xqt = nc.dram_tensor("xqt", [D, S], BF16, kind="ExternalInput").ap()
    xkt = nc.dram_tensor("xkt", [D, S], BF16, kind="ExternalInput").ap()
    xvt = nc.dram_tensor("xvt", [D, S], BF16, kind="ExternalInput").ap()
    wqt = nc.dram_tensor("wqt", [D, D], BF16, kind="ExternalInput").ap()
    wkt = nc.dram_tensor("wkt", [D, D], BF16, kind="ExternalInput").ap()
    wvt = nc.dram_tensor("wvt", [D, D], BF16, kind="ExternalInput").ap()
    wft = nc.dram_tensor("wft", [D, D], BF16, kind="ExternalInput").ap()
    out = nc.dram_tensor("out", [S, D], F32, kind="ExternalOutput").ap()

    from contextlib import ExitStack

    with tile.TileContext(nc) as tc:
        with (
            tc.tile_pool(name="persist", bufs=1) as pp,
        ):
            # v natural layout, ones column after each head (softmax denom)
            vpv = [pp.tile([P, H * (DK + 1)], BF16, tag=f"v{t}", name=f"v{t}")
                   for t in range(NT)]
            ctxT = [pp.tile([P, S], BF16, tag=f"c{t}", name=f"c{t}")
                    for t in range(NT)]

            with ExitStack() as stk:
                ap_ = stk.enter_context(tc.tile_pool(name="attn", bufs=2))
                xtq = [ap_.tile([P, S], BF16, tag="xtq", name="xtq", bufs=8)
                       for _ in range(NT)]
                xtk = [ap_.tile([P, S], BF16, tag="xtk", name="xtk", bufs=8)
                       for _ in range(NT)]
                # fc weights for ct 0-3 get their own slots so the early fc
                # chunks can run while xtq is still live
                wf4 = [ap_.tile([P, S], BF16, tag="wf4", name="wf4", bufs=4)
                       for _ in range(4)]

                # ---- v projection first (attention needs all of v) ----
                with tc.tile_pool(name="vld", bufs=8) as vp, \
                     tc.tile_pool(name="vps", bufs=8, space="PSUM") as vpsp:
                    xts = [vp.tile([P, S], BF16, tag="xt", name="xt")
                           for _ in range(NT)]
                    ws = [vp.tile([P, D], BF16, tag="w", name="w")
                          for _ in range(NT)]
                    # v inputs chunked + d-ordered, split across BOTH
                    # hwdge issue queues (sync + scalar) so they land first;
                    # q/k follow, fc weights last (needed only at iter 6)
                    for t in range(NT):
                        nc.sync.dma_start(
                            out=xts[t][0:64, :],
                            in_=xvt[t * P:t * P + 64, :])
                        nc.scalar.dma_start(
                            out=xts[t][64:128, :],
                            in_=xvt[t * P + 64:(t + 1) * P, :])
                        nc.sync.dma_start(
                            out=ws[t][0:64, :],
                            in_=wvt[t * P:t * P + 64, :])
                        nc.scalar.dma_start(
                            out=ws[t][64:128, :],
                            in_=wvt[t * P + 64:(t + 1) * P, :])
                    for t in range(NT):
                        enq = nc.sync if t % 2 == 0 else nc.scalar
                        enk = nc.scalar if t % 2 == 0 else nc.sync
                        enq.dma_start(out=xtq[t][:],
                                      in_=xqt[t * P:(t + 1) * P, :])
                        enk.dma_start(out=xtk[t][:],
                                      in_=xkt[t * P:(t + 1) * P, :])
                    for ct in range(4):
                        nc.scalar.dma_start(out=wf4[ct][:],
                                            in_=wft[ct * P:(ct + 1) * P, :])

                    for s2 in range(NT):
                        pss = [vpsp.tile([P, CH], F32, tag="vp", name="vp")
                               for _ in range(NCH)]
                        for d in range(NT):
                            for c in range(NCH):
                                nc.tensor.matmul(
                                    pss[c][:],
                                    lhsT=xts[d][:, s2 * P:(s2 + 1) * P],
                                    rhs=ws[d][:, c * CH:(c + 1) * CH],
                                    start=(d == 0),
                                    stop=(d == NT - 1),
                                )
                        nc.vector.memset(
                            vpv[s2][:, 0:H * 65].rearrange(
                                "p (h x) -> p h x", x=65)[:, :, 64:65],
                            1.0,
                        )
                        for c in range(NCH):
                            dst_ap = vpv[s2][:, c * 520:(c + 1) * 520].rearrange(
                                "p (h x) -> p h x", x=65)[:, :, 0:64]
                            src_ap = pss[c][:].rearrange(
                                "p (h x) -> p h x", x=64)
                            nc.vector.tensor_copy(dst_ap, src_ap)

                # second SBUF pool for tags that only exist after the
                # v-load pool is gone (exp tiles, recip rows, fc partials) —
                # keeps the peak footprint under the SBUF limit
                ap2 = stk.enter_context(tc.tile_pool(name="attn2", bufs=2))
                # main psum pool for the attention loop: proj 2 + sc 4 + pv 2
                psp = stk.enter_context(
                    tc.tile_pool(name="psum", bufs=2, space="PSUM"))

                def qkproj(a):
                    # q/k head-pair tiles rotate (lifetime: this iteration's
                    # projection + next iteration's scores)
                    outs = []
                    for xts_, wsrc, tg in ((xtq, wqt, "qTr"), (xtk, wkt, "kTr")):
                        dst = pp.tile([P, S], BF16, tag=tg, name=tg, bufs=3)
                        pss = [psp.tile([P, CH], F32, tag="proj", name="proj")
                               for _ in range(NCH)]
                        for d in range(NT):
                            wt = ap_.tile([P, P], BF16, tag="wqk", name="wqk",
                                          bufs=12)
                            nc.sync.dma_start(
                                out=wt[:],
                                in_=wsrc[d * P:(d + 1) * P, a * P:(a + 1) * P])
                            for c in range(NCH):
                                nc.tensor.matmul(
                                    pss[c][:],
                                    lhsT=wt[:],
                                    rhs=xts_[d][:, c * CH:(c + 1) * CH],
                                    start=(d == 0),
                                    stop=(d == NT - 1),
                                )
                        for c in range(NCH):
                            nc.vector.tensor_copy(
                                dst[:, c * CH:(c + 1) * CH], pss[c][:])
                        outs.append(dst)
                    return outs

                def scores(qk):
                    qTa, kTa = qk
                    # per (s2, c): one fp32 psum tile [128, 1024] spanning 2
                    # banks; the two heads' K=64 matmuls (N=512 each, row
                    # strips 0-63 / 64-127) run concurrently and each fill
                    # one bank; a single fused exp [128, 1024] reads both.
                    exps = []
                    for s2 in range(NT):
                        ecs = []
                        for c in range(NCH):
                            sc = psp.tile([P, S], F32, tag="sc", name="sc")
                            for g in range(2):
                                nc.tensor.matmul(
                                    sc[:, g * CH:(g + 1) * CH],
                                    lhsT=kTa[g * DK:(g + 1) * DK,
                                             s2 * P:(s2 + 1) * P],
                                    rhs=qTa[g * DK:(g + 1) * DK,
                                            c * CH:(c + 1) * CH],
                                    start=True, stop=True,
                                    tile_position=(g * DK, 0),
                                )
                            ec = ap2.tile([P, S], BF16, tag=f"e{s2}c{c}",
                                          name=f"e{s2}c{c}")
                            nc.scalar.activation(ec[:], sc[:], EXP,
                                                 scale=0.125)
                            ecs.append(ec)
                        exps.append(ecs)
                    return exps

                def pv_phase(a, exps, on_act=False):
                    # PV accumulation; ctx_unnorm copied straight out of
                    # psum; the denominator row feeds reciprocal_approx_fast
                    # directly (no gather), results land in r0 at
                    # partition 0 laid out [g0c0|g0c1|g1c0|g1c1].
                    r0 = ap2.tile([1, 2 * S], F32, tag="r0", name="r0",
                                  bufs=1)
                    for g in range(2):
                        h = 2 * a + g
                        pvs = [psp.tile([P, CH], F32, tag="pv", name="pv")
                               for _ in range(NCH)]
                        for c in range(NCH):
                            for s2 in range(NT):
                                nc.tensor.matmul(
                                    pvs[c][0:DK + 1, :],
                                    lhsT=vpv[s2][:, h * 65:(h + 1) * 65],
                                    rhs=exps[s2][c][:, g * CH:(g + 1) * CH],
                                    start=(s2 == 0),
                                    stop=(s2 == NT - 1),
                                )
                        for c in range(NCH):
                            cp = nc.scalar.copy if on_act else \
                                nc.vector.tensor_copy
                            cp(ctxT[a][g * DK:(g + 1) * DK,
                                       c * CH:(c + 1) * CH],
                               pvs[c][0:DK, :])
                            cp(r0[0:1, g * S + c * CH:
                                  g * S + (c + 1) * CH],
                               pvs[c][DK:DK + 1, :])
                    nc.vector.reciprocal_approx_fast(out=r0[:], in_=r0[:])
                    r0b = ap2.tile([1, 2 * S], BF16, tag="r0b", name="r0b",
                                   bufs=2)
                    nc.vector.tensor_copy(r0b[:], r0[:])
                    return r0b

                def norm_phase(a, r0b):
                    # broadcast recip rows across partitions (one per head)
                    # and scale ctxT with one bf16 multiply per head
                    for g in range(2):
                        rb = ap2.tile([P, S], BF16, tag="rb", name="rb",
                                      bufs=2)
                        nc.gpsimd.partition_broadcast(
                            rb[:], r0b[0:1, g * S:(g + 1) * S])
                        sl = ctxT[a][g * DK:(g + 1) * DK, :]
                        nc.vector.tensor_mul(sl, sl, rb[g * DK:(g + 1) * DK, :])

                # fc partial tiles (bf16) hold ct0-3, later += ct4-6
                fcp = [ap2.tile([P, CH], BF16, tag=f"fp{i}", name=f"fp{i}",
                                bufs=1)
                       for i in range(2 * NT)]

                def fc_chunk(s1, c, cts, wfs, first):
                    pss = psp.tile([P, CH], F32, tag="proj", name="proj")
                    for i, ct in enumerate(cts):
                        nc.tensor.matmul(
                            pss[:],
                            lhsT=ctxT[ct][:, s1 * P:(s1 + 1) * P],
                            rhs=wfs[ct][:, c * CH:(c + 1) * CH],
                            start=(i == 0),
                            stop=(i == len(cts) - 1),
                        )
                    if first:
                        nc.vector.tensor_copy(fcp[s1 * NCH + c][:], pss[:])
                    else:
                        nc.vector.tensor_add(
                            fcp[s1 * NCH + c][:], fcp[s1 * NCH + c][:],
                            pss[:])

                exps_hist = None
                qk_hist = None
                r0_hist = {}
                for a in range(NT):
                    qk_new = qkproj(a)
                    if a >= 2:
                        r0_hist[a - 2] = pv_phase(a - 2, exps_hist)
                    if a >= 1:
                        exps_hist = scores(qk_hist)
                    qk_hist = qk_new
                    if a >= 3:
                        norm_phase(a - 3, r0_hist.pop(a - 3))
                    # fc for ct0-3 interleaves with the last two iterations
                    if a == 6:
                        for s1 in range(4):
                            for c in range(NCH):
                                fc_chunk(s1, c, [0, 1, 2, 3], wf4, True)
                    if a == 7:
                        for s1 in range(4, NT):
                            for c in range(NCH):
                                fc_chunk(s1, c, [0, 1, 2, 3], wf4, True)

                # fc weights for ct 4-7 reuse the q-input slots freed after
                # the last projection
                wf = {}
                for ct in range(4, NT):
                    t_ = ap_.tile([P, S], BF16, tag="xtq", name="wf", bufs=8)
                    nc.sync.dma_start(out=t_[:],
                                      in_=wft[ct * P:(ct + 1) * P, :])
                    wf[ct] = t_

                # drain: sc(7); pv(6)+norm(6); fc ct4-6; pv(7)+norm(7);
                # fc ct7 + final add + store
                exps_last = scores(qk_hist)
                norm_phase(NT - 3, r0_hist.pop(NT - 3))
                r0 = pv_phase(NT - 2, exps_hist)
                norm_phase(NT - 2, r0)
                for s1 in range(NT):
                    for c in range(NCH):
                        fc_chunk(s1, c, [4, 5, 6], wf, False)
                r0 = pv_phase(NT - 1, exps_last, on_act=True)
                norm_phase(NT - 1, r0)

                for s1 in range(NT):
                    for c in range(NCH):
                        pss = psp.tile([P, CH], F32, tag="proj", name="proj")
                        nc.tensor.matmul(
                            pss[:],
                            lhsT=ctxT[NT - 1][:, s1 * P:(s1 + 1) * P],
                            rhs=wf[NT - 1][:, c * CH:(c + 1) * CH],
                            start=True, stop=True,
                        )
                        ob = ap_.tile([P, CH], F32, tag="xtk", name="ob",
                                      bufs=8)
                        nc.vector.tensor_add(
                            ob[:], fcp[s1 * NCH + c][:], pss[:])
                        for hh in range(2):
                            eng = nc.sync if hh == 0 else nc.scalar
                            eng.dma_start(
                                out=out[s1 * P + hh * 64:
                                        s1 * P + (hh + 1) * 64,
                                        c * CH:(c + 1) * CH],
                                in_=ob[hh * 64:(hh + 1) * 64, :],
                            )

    nc.compile()
    return nc


def run(inputs, trace=False):
    """inputs: dict with Q,K,V [8,1024,1024] and WQ,WK,WV,Wfc [1024,1024].
    Returns (out [8,1024,1024] fp32, exec_time_ns or None)."""
    if "nc" not in _CACHE:
        _CACHE["nc"] = _build()
    nc = _CACHE["nc"]

    import ml_dtypes
    bf16 = ml_dtypes.bfloat16
    f32 = np.float32
    wqt = np.ascontiguousarray(np.asarray(inputs["WQ"], dtype=f32).T.astype(bf16))
    wkt = np.ascontiguousarray(np.asarray(inputs["WK"], dtype=f32).T.astype(bf16))
    wvt = np.ascontiguousarray(np.asarray(inputs["WV"], dtype=f32).T.astype(bf16))
    wft = np.ascontiguousarray(np.asarray(inputs["Wfc"], dtype=f32).T.astype(bf16))
    Q = np.asarray(inputs["Q"], dtype=f32)
    K = np.asarray(inputs["K"], dtype=f32)
    V = np.asarray(inputs["V"], dtype=f32)

    in_maps = [
        {
            "xqt": np.ascontiguousarray(Q[b].T.astype(bf16)),
            "xkt": np.ascontiguousarray(K[b].T.astype(bf16)),
            "xvt": np.ascontiguousarray(V[b].T.astype(bf16)),
            "wqt": wqt, "wkt": wkt, "wvt": wvt, "wft": wft,
        }
        for b in range(8)
    ]
    res = run_bass_kernel_spmd(nc, in_maps, core_ids=list(range(8)), trace=trace)
    out = np.stack([res.results[b]["out"] for b in range(8)], axis=0)
    return out.astype(np.float32), res.exec_time_ns


def kernel(**inputs):
    return run(inputs, trace=False)[0]
